# revision 15
# baseline (speedup 1.0000x reference)
"""GAT spatio-temporal model Trainium2 kernel (v4).

Sharding: data-parallel over batch B=8 -> 8 NeuronCores (1 graph each).

v4 core trick: exp(leaky_relu(s1[n]+s2[m])) = max(E1*E2, E1a*E2a) with
E = exp(s), Ea = exp(alpha*s) (exp monotone, lrelu(x) = max(x, a*x)).
Factor p = E1a[n] * E2[m] * max(E1b[n], E2inv[m]) with E1b = exp((1-a)s1),
E2inv = exp(-(1-a)s2).  E1a[n] is constant along the softmax axis (m) and
cancels; E2[m] folds into the den / AV matmul lhsT weights.  The whole
[N,N] attention tensor is then ONE fused DVE op per 128-chunk:
scalar_tensor_tensor(out, E1b_bcast, E2inv_col, maskT, max, mult).
No N^2 ScalarE work; s1/s2 come from one matmul via precomposed W@a.
All N^2 matmuls bf16; LN matmuls float32r.

Shapes (hardcoded): B=8, N=512, Din=64, H=8, F=128, L=2.
"""
import os
import numpy as np
from contextlib import ExitStack

import concourse.bass as bass
import concourse.tile as tile
from concourse import bacc, mybir
from concourse.bass_utils import run_bass_kernel_spmd
from concourse.masks import make_identity

F32 = mybir.dt.float32
F32R = mybir.dt.float32r
BF16 = mybir.dt.bfloat16
AF = mybir.ActivationFunctionType
OP = mybir.AluOpType

B, N, DIN, H, F, L = 8, 512, 64, 8, 128, 2
NCHUNK = N // 128  # 4
ALPHA = 0.2
BETA = 1.0 - ALPHA
LN_EPS = 1e-5

GP_STT = int(os.environ.get("K_GP_STT", "0"))   # STT chunks on gpsimd
GP_ELU = os.environ.get("K_GP_ELU", "0") == "1"  # ELU tensor_scalar on gpsimd
GP_EMAX = os.environ.get("K_GP_EMAX", "0") == "1"  # ELU max on gpsimd
GP_PSC = int(os.environ.get("K_GP_PSC", "0"))   # projNp scale chunks on gpsimd

_CACHE = {}


def _bcast_row(ap_row):
    return bass.AP(tensor=ap_row.tensor, offset=ap_row.offset, ap=[[0, 128], [1, N]])


def _r(ap):
    return ap.bitcast(F32R)


def build_nc():
    nc = bacc.Bacc("TRN2", target_bir_lowering=False, debug=False)

    x_d = nc.dram_tensor("x", [N, DIN], F32, kind="ExternalInput").ap()
    adj_d = nc.dram_tensor("adj", [N, N], mybir.dt.int32, kind="ExternalInput").ap()
    Wp_d = nc.dram_tensor("Wp", [DIN, F], F32, kind="ExternalInput").ap()
    bp_d = nc.dram_tensor("bp", [F], F32, kind="ExternalInput").ap()
    Wh_d = nc.dram_tensor("W_heads", [L, H, F, F], F32, kind="ExternalInput").ap()
    ah_d = nc.dram_tensor("a_heads", [L, H, 2 * F], F32, kind="ExternalInput").ap()
    Wo_d = nc.dram_tensor("W_out", [L, H * F, F], F32, kind="ExternalInput").ap()
    ao_d = nc.dram_tensor("a_out", [L, 2 * F], F32, kind="ExternalInput").ap()
    g_d = nc.dram_tensor("ln_g", [L, F], F32, kind="ExternalInput").ap()
    b_d = nc.dram_tensor("ln_b", [L, F], F32, kind="ExternalInput").ap()
    out_d = nc.dram_tensor("out", [N, F], F32, kind="ExternalOutput").ap()
    # DRAM bounce buffers: E1b rows (per layer) + per-head recip rows
    ebl_d = [nc.dram_tensor(f"eblk{l}", [16, N], BF16, kind="ExternalOutput").ap()
             for l in range(L)]
    scr_d = [nc.dram_tensor(f"scratch{i}", [1, N], BF16, kind="ExternalOutput").ap()
             for i in range(18)]

    with tile.TileContext(nc) as tc, ExitStack() as ctx:
        const = ctx.enter_context(tc.tile_pool(name="const", bufs=1))
        sx = ctx.enter_context(tc.tile_pool(name="sx", bufs=2))
        sproj = ctx.enter_context(tc.tile_pool(name="sproj", bufs=10))
        sbcast = ctx.enter_context(tc.tile_pool(name="sbcast", bufs=10))
        sexp = ctx.enter_context(tc.tile_pool(name="sexp", bufs=4))
        smulti = ctx.enter_context(tc.tile_pool(name="smulti", bufs=9))
        sbig = ctx.enter_context(tc.tile_pool(name="sbig", bufs=3))
        srow = ctx.enter_context(tc.tile_pool(name="srow", bufs=5))
        shd = ctx.enter_context(tc.tile_pool(name="shd", bufs=4))
        smask = ctx.enter_context(tc.tile_pool(name="smask", bufs=4))
        pou = ctx.enter_context(tc.tile_pool(name="pou", bufs=3, space="PSUM"))
        pmisc = ctx.enter_context(tc.tile_pool(name="pmisc", bufs=2, space="PSUM"))
        prow = ctx.enter_context(tc.tile_pool(name="prow", bufs=3, space="PSUM"))

        # ---------------- constants ----------------
        ones_row = const.tile([1, N], F32)
        nc.vector.memset(ones_row, 1.0)
        ones_row_bf = const.tile([1, N], BF16)
        nc.vector.memset(ones_row_bf, 1.0)
        ones_col = const.tile([128, 1], F32)
        nc.vector.memset(ones_col, 1.0)
        ones_col_bf = const.tile([128, 1], BF16)
        nc.vector.memset(ones_col_bf, 1.0)
        ident = const.tile([128, 128], F32)
        make_identity(nc, ident)
        ident_bf = const.tile([128, 128], BF16)
        nc.vector.tensor_copy(ident_bf, ident)
        eps1 = const.tile([1, 1], F32)
        nc.vector.memset(eps1, LN_EPS)

        Wp_sb = const.tile([DIN, F], F32)
        nc.sync.dma_start(Wp_sb, Wp_d)
        bp_col = const.tile([F, 1], F32)
        nc.sync.dma_start(bp_col, bp_d.rearrange("(f one) -> f one", one=1))
        x_chunks = []
        for c in range(NCHUNK):
            xc = shd.tile([128, DIN], F32, tag="xchunk")
            nc.sync.dma_start(xc, x_d[bass.ts(c, 128), :])
            x_chunks.append(xc)

        # per-layer weight loads: gpsimd swdge DMAs cast f32->bf16 directly
        Wh_ball = [const.tile([F, H, F], BF16, name=f"WhB{l}") for l in range(L)]
        for l in range(L):
            nc.gpsimd.dma_start(Wh_ball[l], Wh_d[l].rearrange("h i o -> i h o"))
        Wh_bf = [[Wh_ball[l][:, h, :] for h in range(H)] for l in range(L)]

        ah_ball = const.tile([F, L * H, 2], BF16)
        nc.gpsimd.dma_start(ah_ball, ah_d.rearrange("l h (t f) -> f (l h) t", t=2))
        ah_bf = [[ah_ball[:, l * H + h, :] for h in range(H)] for l in range(L)]

        Wo_ball = [const.tile([128, H, F], BF16, name=f"WoB{l}") for l in range(L)]
        for l in range(L):
            nc.gpsimd.dma_start(Wo_ball[l], Wo_d[l].rearrange("(c p) f -> p c f", p=128))
        Wo_bf = Wo_ball

        ao_ball = const.tile([F, L, 2], BF16)
        nc.gpsimd.dma_start(ao_ball, ao_d.rearrange("l (t f) -> f l t", t=2))
        ao_bf = [ao_ball[:, l, :] for l in range(L)]

        g_all = const.tile([1, L, F], F32)
        nc.scalar.dma_start(g_all, g_d.rearrange("l f -> (l f)").rearrange(
            "(one l f) -> one l f", one=1, l=L))
        b_all = const.tile([1, L, F], F32)
        nc.scalar.dma_start(b_all, b_d.rearrange("l f -> (l f)").rearrange(
            "(one l f) -> one l f", one=1, l=L))
        gc_all = const.tile([F, L], F32)
        nc.scalar.dma_start(gc_all, g_d.rearrange("l f -> f l"))
        g_row = [g_all[:, l, :] for l in range(L)]
        b_row = [b_all[:, l, :] for l in range(L)]
        g_col = [gc_all[:, l:l + 1] for l in range(L)]
        gb_bf = const.tile([1, 2 * L, F], BF16)
        nc.vector.tensor_copy(gb_bf[:, 0:L, :], g_all)
        nc.vector.tensor_copy(gb_bf[:, L:2 * L, :], b_all)
        g_row_bf = [gb_bf[:, l, :] for l in range(L)]
        b_row_bf = [gb_bf[:, L + l, :] for l in range(L)]

        # ------------- WhT (transposed head weights) + Wtilde = W @ a -------
        WhT_ball = [const.tile([F, H, F], BF16, name=f"WhT{l}") for l in range(L)]
        for l in range(L):
            for h in range(H):
                pt = pou.tile([128, 128], BF16, tag="oU")
                nc.tensor.transpose(pt, Wh_bf[l][h], ident_bf)
                if h % 2 == 0:
                    nc.scalar.activation(WhT_ball[l][:, h, :], pt, AF.Copy)
                else:
                    nc.vector.tensor_copy(WhT_ball[l][:, h, :], pt)
        Wt_bf = [const.tile([F, 2 * H], BF16, name=f"Wt{l}") for l in range(L)]
        for l in range(L):
            pw = prow.tile([128, 2 * H], F32, tag="prow")
            for h in range(H):
                nc.tensor.matmul(pw[:, 2 * h:2 * h + 2], WhT_ball[l][:, h, :],
                                 ah_bf[l][h], start=True, stop=True)
            nc.scalar.activation(Wt_bf[l], pw, AF.Copy)

        # ---------------- x -> xT, input projection ----------------
        xT = const.tile([DIN, N], F32)
        ph = pmisc.tile([128, N], F32, tag="pbig")
        hT = sbig.tile([128, N], F32, tag="hT")
        hT_bf = sbig.tile([128, N], BF16, tag="hTb", bufs=2)
        for c in range(NCHUNK):
            pt = pmisc.tile([DIN, 128], F32, tag="pbig")
            nc.tensor.transpose(pt, x_chunks[c], ident)
            nc.scalar.activation(xT[:, bass.ts(c, 128)], pt, AF.Copy)
            nc.tensor.matmul(ph[:, bass.ts(c, 128)], Wp_sb, xT[:, bass.ts(c, 128)],
                             start=True, stop=True)
            nc.scalar.activation(hT[:, bass.ts(c, 128)], ph[:, bass.ts(c, 128)],
                                 AF.Relu, bias=bp_col)
            nc.vector.tensor_copy(hT_bf[:, bass.ts(c, 128)], hT[:, bass.ts(c, 128)])

        # ---------------- adj -> maskT (bf16, transposed) ----------------
        adj_f = []
        for r in range(NCHUNK):
            ai = shd.tile([128, N], mybir.dt.int32, tag="adji")
            nc.scalar.dma_start(ai, adj_d[bass.ts(r, 128), :])
            af = smask.tile([128, N], BF16, tag="adjf")
            nc.vector.tensor_copy(af, ai)
            adj_f.append(af)
        maskT = [const.tile([128, N], BF16, name=f"maskT{c}") for c in range(NCHUNK)]
        for r in range(NCHUNK):
            for c in range(NCHUNK):
                pm = pmisc.tile([128, 128], BF16, tag="pbig")
                nc.tensor.transpose(pm, adj_f[r][:, bass.ts(c, 128)], ident_bf)
                nc.scalar.activation(maskT[c][:, bass.ts(r, 128)], pm, AF.Copy)

        # ------------- attention body (shared by heads & out-att) -----------
        def attention(e1b_sb, e2i_cols, e2_cols, projNp, hid, out_f32=False):
            """e1b_sb: [128,N] bf16 bcast of E1b row.  e2i_cols/e2_cols: 4
            [128,1] col APs (E2inv f32 / E2 bf16).  projNp: [128,NCHUNK,128]
            bf16 AV lhsT already scaled by E2[m].  Returns outT = pou/den."""
            t_m = sexp.tile([128, NCHUNK, N], BF16, tag="t_m", bufs=3)
            s_t = sexp.tile([128, NCHUNK, N], BF16, tag="s_t", bufs=5)
            for c in range(NCHUNK):
                nc.vector.tensor_scalar_max(t_m[:, c, :], e1b_sb, e2i_cols[c])
                nc.vector.tensor_tensor(s_t[:, c, :], t_m[:, c, :], maskT[c],
                                        OP.mult)
            den_ps = prow.tile([1, N], F32, tag="prow")
            for c in range(NCHUNK):
                nc.tensor.matmul(den_ps, e2_cols[c], s_t[:, c, :],
                                 start=(c == 0), stop=(c == NCHUNK - 1))
            pou_ps = pou.tile([128, N], F32, tag="oU")
            for c in range(NCHUNK):
                nc.tensor.matmul(pou_ps, projNp[:, c, :], s_t[:, c, :],
                                 start=(c == 0), stop=(c == NCHUNK - 1))
            rrow = srow.tile([1, N], F32, tag="rrowf")
            nc.vector.reciprocal_approx_fast(rrow, den_ps)
            rrow_bf = srow.tile([1, N], BF16, tag="rrowb")
            nc.vector.tensor_copy(rrow_bf, rrow)
            nc.sync.dma_start(scr_d[hid], rrow_bf)
            rep = sbcast.tile([128, N], BF16, tag="rep", bufs=6)
            nc.sync.dma_start(rep, _bcast_row(scr_d[hid][0, :]))
            pou_bf = shd.tile([128, N], BF16, tag="poubf")
            nc.scalar.activation(pou_bf, pou_ps, AF.Copy)
            outT = sbig.tile([128, N], F32 if out_f32 else BF16, tag="outT",
                             bufs=4)
            nc.vector.tensor_tensor(outT, pou_bf, rep, OP.mult)
            return outT

        # ---------------- layers ----------------
        for l in range(L):
            residT = hT
            # --- rows for all heads: s12[2h] = s1_h, s12[2h+1] = s2_h
            s12_ps = prow.tile([2 * H, N], F32, tag="prow")
            nc.tensor.matmul(s12_ps, Wt_bf[l], hT_bf, start=True, stop=True)
            Eblk = sx.tile([16, N], BF16, tag="Eblk")   # exp(+beta*s): rows 2h = E1b
            nc.scalar.activation(Eblk, s12_ps, AF.Exp, scale=BETA)
            Xneg = sx.tile([16, N], BF16, tag="Xneg")   # exp(-beta*s): 2h+1 = E2inv
            nc.scalar.activation(Xneg, s12_ps, AF.Exp, scale=-BETA)
            Xpos = sx.tile([16, N], BF16, tag="Xpos")   # exp(s): 2h+1 = E2
            nc.scalar.activation(Xpos, s12_ps, AF.Exp, scale=1.0)
            # E1b broadcasts: one DRAM bounce write of all rows, then one
            # stride-0 broadcast read per head, spread across DMA queues
            dmaq = [nc.sync, nc.scalar, nc.gpsimd]
            nc.sync.dma_start(ebl_d[l], Eblk)
            e1b = []
            for h in range(H):
                row = ebl_d[l][2 * h, :]
                src_bc = bass.AP(tensor=row.tensor, offset=row.offset,
                                 ap=[[0, 128], [1, N]])
                eb = sbcast.tile([128, N], BF16, tag="e1b")
                dmaq[h % 3].dma_start(eb, src_bc)
                e1b.append(eb)
            # --- columns: transpose Xneg/Xpos -> Xcols [128, 8*16]
            xc_ps = prow.tile([128, 8 * 16], BF16, tag="prow")
            for c in range(NCHUNK):
                nc.tensor.transpose(xc_ps[:, c * 16:(c + 1) * 16],
                                    Xneg[:, bass.ts(c, 128)], ident_bf[0:16, 0:16])
                nc.tensor.transpose(xc_ps[:, 64 + c * 16:64 + (c + 1) * 16],
                                    Xpos[:, bass.ts(c, 128)], ident_bf[0:16, 0:16])
            Xcols = sx.tile([128, 8 * 16], F32, tag="Xcols")
            nc.scalar.activation(Xcols, xc_ps, AF.Copy)
            Xcols_bf = sx.tile([128, 8 * 16], BF16, tag="Xcolsb")
            nc.vector.tensor_copy(Xcols_bf, xc_ps)

            def e2i_col(h, c):
                j = c * 16 + 2 * h + 1
                return Xcols[:, j:j + 1]

            def e2_col(h, c):
                j = 64 + c * 16 + 2 * h + 1
                return Xcols[:, j:j + 1]

            def e2_col_bf(h, c):
                j = 64 + c * 16 + 2 * h + 1
                return Xcols_bf[:, j:j + 1]

            # --- projN per head (scaled by E2[m])
            projNp = []
            for h in range(H):
                pN = pmisc.tile([128, N], F32, tag="pbig")
                for c in range(NCHUNK):
                    nc.tensor.matmul(pN[:, bass.ts(c, 128)], hT_bf[:, bass.ts(c, 128)],
                                     Wh_bf[l][h], start=True, stop=True)
                pp = sproj.tile([128, NCHUNK, 128], BF16, tag="projNp")
                for c in range(NCHUNK):
                    nc.scalar.activation(pp[:, c, :], pN[:, bass.ts(c, 128)],
                                         AF.Identity, scale=e2_col(h, c))
                projNp.append(pp)
            # --- attention per head + ELU
            multiT = []
            for h in range(H):
                outT = attention(
                    e1b[h],
                    [e2i_col(h, c) for c in range(NCHUNK)],
                    [e2_col_bf(h, c) for c in range(NCHUNK)],
                    projNp[h], l * 9 + h)
                ex = shd.tile([128, N], BF16, tag="elu_ex")
                nc.scalar.activation(ex, outT, AF.Exp)
                eng = nc.gpsimd if GP_ELU else nc.vector
                eng.tensor_scalar(ex, ex, 1.0, -1.0, OP.min, OP.add)
                mh = smulti.tile([128, N], BF16, tag="multi")
                eng2 = nc.gpsimd if GP_EMAX else nc.vector
                eng2.tensor_tensor(mh, outT, ex, OP.max)
                multiT.append(mh)

            # --- W_out projection
            ph2 = pou.tile([128, N], F32, tag="oU")
            for h in range(H):
                nc.tensor.matmul(ph2, Wo_bf[l][:, h, :], multiT[h],
                                 start=(h == 0), stop=(h == H - 1))
            h2_bf = sbig.tile([128, N], BF16, tag="h2b", bufs=2)
            nc.scalar.activation(h2_bf, ph2, AF.Copy)

            # --- single out-attention
            s12o_ps = prow.tile([2, N], F32, tag="prow")
            nc.tensor.matmul(s12o_ps, ao_bf[l], h2_bf, start=True, stop=True)
            Xo_b = sx.tile([2, N], BF16, tag="Xo_b")    # row 0 = E1b_o
            nc.scalar.activation(Xo_b, s12o_ps, AF.Exp, scale=BETA)
            Xo_nb = sx.tile([2, N], BF16, tag="Xo_nb")  # row 1 = E2inv_o
            nc.scalar.activation(Xo_nb, s12o_ps, AF.Exp, scale=-BETA)
            Xo_1 = sx.tile([2, N], BF16, tag="Xo_1")    # row 1 = E2_o
            nc.scalar.activation(Xo_1, s12o_ps, AF.Exp, scale=1.0)
            xo_ps = prow.tile([128, 16], BF16, tag="prow")
            for c in range(NCHUNK):
                nc.tensor.transpose(xo_ps[:, c * 2:(c + 1) * 2],
                                    Xo_nb[:, bass.ts(c, 128)], ident_bf[0:2, 0:2])
                nc.tensor.transpose(xo_ps[:, 8 + c * 2:8 + (c + 1) * 2],
                                    Xo_1[:, bass.ts(c, 128)], ident_bf[0:2, 0:2])
            Xoc = sx.tile([128, 16], F32, tag="Xoc")
            nc.scalar.activation(Xoc, xo_ps, AF.Copy)
            Xoc_bf = sx.tile([128, 16], BF16, tag="Xocb")
            nc.vector.tensor_copy(Xoc_bf, xo_ps)
            # E1b_o broadcast via PE rank-1 (low latency; PE idle here)
            ebo_ps = pmisc.tile([128, N], F32, tag="pbig")
            nc.tensor.matmul(ebo_ps, ones_row_bf[:, 0:128], Xo_b[0:1, :], start=True, stop=True)
            e1bo = sbcast.tile([128, N], BF16, tag="e1b")
            nc.scalar.activation(e1bo, ebo_ps, AF.Copy)
            # h2N via transposes, scaled by E2o[m]
            h2n_ps = pmisc.tile([128, N], BF16, tag="pbig")
            for c in range(NCHUNK):
                nc.tensor.transpose(h2n_ps[:, bass.ts(c, 128)],
                                    h2_bf[:, bass.ts(c, 128)], ident_bf)
            h2Np = sproj.tile([128, NCHUNK, 128], BF16, tag="projNp")
            for c in range(NCHUNK):
                nc.scalar.activation(h2Np[:, c, :], h2n_ps[:, bass.ts(c, 128)],
                                     AF.Identity, scale=Xoc[:, 8 + c * 2 + 1:8 + c * 2 + 2])
            outsT = attention(
                e1bo,
                [Xoc[:, c * 2 + 1:c * 2 + 2] for c in range(NCHUNK)],
                [Xoc_bf[:, 8 + c * 2 + 1:8 + c * 2 + 2] for c in range(NCHUNK)],
                h2Np, l * 9 + 8, out_f32=True)

            # ---- residual + LN over partition dim ----
            xs = sbig.tile([128, N], F32, tag="xs", bufs=2)
            nc.vector.tensor_tensor(xs, outsT, residT, OP.add)
            xs_bf = sbig.tile([128, N], BF16, tag="xsqb", bufs=2)
            nc.vector.tensor_copy(xs_bf, xs)
            xsq = sbig.tile([128, N], BF16, tag="xsq", bufs=2)
            nc.scalar.activation(xsq, xs, AF.Square)
            pmu = prow.tile([1, N], F32, tag="prow")
            nc.tensor.matmul(pmu, ones_col_bf, xs_bf, start=True, stop=True)
            psq = prow.tile([1, N], F32, tag="prow")
            nc.tensor.matmul(psq, ones_col_bf, xsq, start=True, stop=True)
            mu = srow.tile([1, N], F32, tag="rowL")
            nc.vector.tensor_scalar_mul(mu, pmu, 1.0 / F)
            msq = srow.tile([1, N], F32, tag="rowL")
            nc.vector.tensor_scalar_mul(msq, psq, 1.0 / F)
            mu2 = srow.tile([1, N], F32, tag="rowL")
            nc.vector.tensor_tensor(mu2, mu, mu, OP.mult)
            var = srow.tile([1, N], F32, tag="rowL")
            nc.vector.tensor_tensor(var, msq, mu2, OP.subtract)
            lnv = srow.tile([1, N], F32, tag="rowL")
            nc.scalar.activation(lnv, var, AF.Ln, bias=eps1)
            rstd = srow.tile([1, N], F32, tag="rowL")
            nc.scalar.activation(rstd, lnv, AF.Exp, scale=-0.5)
            mr = srow.tile([1, N], F32, tag="rowL")
            nc.vector.tensor_tensor(mr, mu, rstd, OP.mult)
            r2 = srow.tile([1, N], BF16, tag="rowLb")
            nc.vector.tensor_scalar_mul(r2, mr, -1.0)
            rstd_bf = srow.tile([1, N], BF16, tag="rowLb")
            nc.vector.tensor_copy(rstd_bf, rstd)
            paff = pmisc.tile([128, N], F32, tag="pbig")
            nc.tensor.matmul(paff, g_row_bf[l], r2, start=True, stop=False)
            nc.tensor.matmul(paff, b_row_bf[l], ones_row_bf, start=False, stop=True)
            prs = pmisc.tile([128, N], F32, tag="pbig")
            nc.tensor.matmul(prs, g_row_bf[l], rstd_bf,
                             start=True, stop=True)
            rep_grstd = sbig.tile([128, N], F32, tag="repo", bufs=2)
            nc.scalar.activation(rep_grstd, prs, AF.Copy)
            y = sbig.tile([128, N], F32, tag="y", bufs=2)
            nc.vector.tensor_tensor(y, xs, rep_grstd, OP.mult)
            hT_new = sbig.tile([128, N], F32, tag="hT")
            nc.vector.tensor_tensor(hT_new, y, paff, OP.add)
            if l < L - 1:
                nc.vector.tensor_scalar_max(hT_new, hT_new, 0.0)
            hT = hT_new
            if l < L - 1:
                hT_bf = sbig.tile([128, N], BF16, tag="hTb", bufs=2)
                nc.vector.tensor_copy(hT_bf, hT)

        # ---------------- output: transpose back ----------------
        for c in range(NCHUNK):
            po = pmisc.tile([128, 128], F32, tag="pbig")
            nc.tensor.transpose(po, hT[:, bass.ts(c, 128)], ident)
            osb = shd.tile([128, 128], F32, tag="osb")
            nc.scalar.activation(osb, po, AF.Copy)
            nc.sync.dma_start(out_d[bass.ts(c, 128), :], osb)

    nc.compile()
    return nc


def _get_nc():
    if "nc" not in _CACHE:
        _CACHE["nc"] = build_nc()
    return _CACHE["nc"]


def kernel(**inputs) -> np.ndarray:
    nc = _get_nc()
    shared = {k: np.ascontiguousarray(np.asarray(inputs[k], dtype=np.float32))
              for k in ("Wp", "bp", "W_heads", "a_heads", "W_out", "a_out",
                        "ln_g", "ln_b")}
    x = np.asarray(inputs["x"], dtype=np.float32)
    adj = np.asarray(inputs["adj"], dtype=np.int32)
    in_maps = [dict(x=np.ascontiguousarray(x[b]),
                    adj=np.ascontiguousarray(adj[b]), **shared)
               for b in range(B)]
    res = run_bass_kernel_spmd(nc, in_maps, core_ids=list(range(B)))
    return np.stack([res.results[b]["out"] for b in range(B)])


if __name__ == "__main__":
    rng = np.random.default_rng(0)
    inputs = dict(
        x=rng.normal(size=(B, N, DIN)).astype(np.float32),
        adj=rng.integers(0, 2, size=(B, N, N)).astype(np.int32),
        Wp=(rng.normal(size=(DIN, F)) * 0.12).astype(np.float32),
        bp=np.zeros(F, dtype=np.float32),
        W_heads=(rng.normal(size=(L, H, F, F)) * 0.08).astype(np.float32),
        a_heads=(rng.normal(size=(L, H, 2 * F)) * 0.08).astype(np.float32),
        W_out=(rng.normal(size=(L, H * F, F)) * 0.03).astype(np.float32),
        a_out=(rng.normal(size=(L, 2 * F)) * 0.08).astype(np.float32),
        ln_g=np.ones((L, F), dtype=np.float32),
        ln_b=np.zeros((L, F), dtype=np.float32),
    )
    out = kernel(**inputs)
    print("out", out.shape, out.dtype, np.abs(out).max())


# revision 16
# speedup vs baseline: 1.0712x; 1.0712x over previous
"""GAT spatio-temporal model Trainium2 kernel (v4).

Sharding: data-parallel over batch B=8 -> 8 NeuronCores (1 graph each).

v4 core trick: exp(leaky_relu(s1[n]+s2[m])) = max(E1*E2, E1a*E2a) with
E = exp(s), Ea = exp(alpha*s) (exp monotone, lrelu(x) = max(x, a*x)).
Factor p = E1a[n] * E2[m] * max(E1b[n], E2inv[m]) with E1b = exp((1-a)s1),
E2inv = exp(-(1-a)s2).  E1a[n] is constant along the softmax axis (m) and
cancels; E2[m] folds into the den / AV matmul lhsT weights.  The whole
[N,N] attention tensor is then ONE fused DVE op per 128-chunk:
scalar_tensor_tensor(out, E1b_bcast, E2inv_col, maskT, max, mult).
No N^2 ScalarE work; s1/s2 come from one matmul via precomposed W@a.
All N^2 matmuls bf16; LN matmuls float32r.

Shapes (hardcoded): B=8, N=512, Din=64, H=8, F=128, L=2.
"""
import os
import numpy as np
from contextlib import ExitStack

import concourse.bass as bass
import concourse.tile as tile
from concourse import bacc, mybir
from concourse.bass_utils import run_bass_kernel_spmd
from concourse.masks import make_identity

F32 = mybir.dt.float32
F32R = mybir.dt.float32r
BF16 = mybir.dt.bfloat16
AF = mybir.ActivationFunctionType
OP = mybir.AluOpType

B, N, DIN, H, F, L = 8, 512, 64, 8, 128, 2
NCHUNK = N // 128  # 4
ALPHA = 0.2
BETA = 1.0 - ALPHA
LN_EPS = 1e-5

GP_STT = int(os.environ.get("K_GP_STT", "0"))   # STT chunks on gpsimd
GP_ELU = os.environ.get("K_GP_ELU", "0") == "1"  # ELU tensor_scalar on gpsimd
GP_EMAX = os.environ.get("K_GP_EMAX", "0") == "1"  # ELU max on gpsimd
GP_PSC = int(os.environ.get("K_GP_PSC", "0"))   # projNp scale chunks on gpsimd

_CACHE = {}


def _bcast_row(ap_row):
    return bass.AP(tensor=ap_row.tensor, offset=ap_row.offset, ap=[[0, 128], [1, N]])


def _r(ap):
    return ap.bitcast(F32R)


def build_nc():
    nc = bacc.Bacc("TRN2", target_bir_lowering=False, debug=False)

    x_d = nc.dram_tensor("x", [N, DIN], F32, kind="ExternalInput").ap()
    adj_d = nc.dram_tensor("adj", [N, N], mybir.dt.int32, kind="ExternalInput").ap()
    Wp_d = nc.dram_tensor("Wp", [DIN, F], F32, kind="ExternalInput").ap()
    bp_d = nc.dram_tensor("bp", [F], F32, kind="ExternalInput").ap()
    Wh_d = nc.dram_tensor("W_heads", [L, H, F, F], F32, kind="ExternalInput").ap()
    ah_d = nc.dram_tensor("a_heads", [L, H, 2 * F], F32, kind="ExternalInput").ap()
    Wo_d = nc.dram_tensor("W_out", [L, H * F, F], F32, kind="ExternalInput").ap()
    ao_d = nc.dram_tensor("a_out", [L, 2 * F], F32, kind="ExternalInput").ap()
    g_d = nc.dram_tensor("ln_g", [L, F], F32, kind="ExternalInput").ap()
    b_d = nc.dram_tensor("ln_b", [L, F], F32, kind="ExternalInput").ap()
    out_d = nc.dram_tensor("out", [N, F], F32, kind="ExternalOutput").ap()
    # DRAM bounce buffers: E1b rows (per layer) + per-head recip rows
    ebl_d = [nc.dram_tensor(f"eblk{l}", [16, N], BF16, kind="ExternalOutput").ap()
             for l in range(L)]
    scr_d = [nc.dram_tensor(f"scratch{i}", [1, N], BF16, kind="ExternalOutput").ap()
             for i in range(18)]

    with tile.TileContext(nc) as tc, ExitStack() as ctx:
        const = ctx.enter_context(tc.tile_pool(name="const", bufs=1))
        sx = ctx.enter_context(tc.tile_pool(name="sx", bufs=2))
        sproj = ctx.enter_context(tc.tile_pool(name="sproj", bufs=10))
        sbcast = ctx.enter_context(tc.tile_pool(name="sbcast", bufs=10))
        sexp = ctx.enter_context(tc.tile_pool(name="sexp", bufs=4))
        smulti = ctx.enter_context(tc.tile_pool(name="smulti", bufs=9))
        sbig = ctx.enter_context(tc.tile_pool(name="sbig", bufs=3))
        srow = ctx.enter_context(tc.tile_pool(name="srow", bufs=5))
        shd = ctx.enter_context(tc.tile_pool(name="shd", bufs=4))
        smask = ctx.enter_context(tc.tile_pool(name="smask", bufs=4))
        pou = ctx.enter_context(tc.tile_pool(name="pou", bufs=3, space="PSUM"))
        pmisc = ctx.enter_context(tc.tile_pool(name="pmisc", bufs=2, space="PSUM"))
        prow = ctx.enter_context(tc.tile_pool(name="prow", bufs=3, space="PSUM"))

        # ---------------- constants ----------------
        ones_row = const.tile([1, N], F32)
        nc.vector.memset(ones_row, 1.0)
        ones_row_bf = const.tile([1, N], BF16)
        nc.vector.memset(ones_row_bf, 1.0)
        ones_col = const.tile([128, 1], F32)
        nc.vector.memset(ones_col, 1.0)
        ones_col_bf = const.tile([128, 1], BF16)
        nc.vector.memset(ones_col_bf, 1.0)
        ident = const.tile([128, 128], F32)
        make_identity(nc, ident)
        ident_bf = const.tile([128, 128], BF16)
        nc.vector.tensor_copy(ident_bf, ident)
        eps1 = const.tile([1, 1], F32)
        nc.vector.memset(eps1, LN_EPS)

        Wp_sb = const.tile([DIN, F], BF16)
        nc.gpsimd.dma_start(Wp_sb, Wp_d)
        bp_col = const.tile([F, 1], F32)
        nc.sync.dma_start(bp_col, bp_d.rearrange("(f one) -> f one", one=1))
        x_chunks = []
        for c in range(NCHUNK):
            xc = shd.tile([128, DIN], F32, tag="xchunk")
            nc.sync.dma_start(xc, x_d[bass.ts(c, 128), :])
            x_chunks.append(xc)

        # per-layer weight loads: gpsimd swdge DMAs cast f32->bf16 directly
        Wh_ball = [const.tile([F, H, F], BF16, name=f"WhB{l}") for l in range(L)]
        for l in range(L):
            nc.gpsimd.dma_start(Wh_ball[l], Wh_d[l].rearrange("h i o -> i h o"))
        Wh_bf = [[Wh_ball[l][:, h, :] for h in range(H)] for l in range(L)]

        ah_ball = const.tile([F, L * H, 2], BF16)
        nc.gpsimd.dma_start(ah_ball, ah_d.rearrange("l h (t f) -> f (l h) t", t=2))
        ah_bf = [[ah_ball[:, l * H + h, :] for h in range(H)] for l in range(L)]

        Wo_ball = [const.tile([128, H, F], BF16, name=f"WoB{l}") for l in range(L)]
        for l in range(L):
            nc.gpsimd.dma_start(Wo_ball[l], Wo_d[l].rearrange("(c p) f -> p c f", p=128))
        Wo_bf = Wo_ball

        ao_ball = const.tile([F, L, 2], BF16)
        nc.gpsimd.dma_start(ao_ball, ao_d.rearrange("l (t f) -> f l t", t=2))
        ao_bf = [ao_ball[:, l, :] for l in range(L)]

        g_all = const.tile([1, L, F], F32)
        nc.scalar.dma_start(g_all, g_d.rearrange("l f -> (l f)").rearrange(
            "(one l f) -> one l f", one=1, l=L))
        b_all = const.tile([1, L, F], F32)
        nc.scalar.dma_start(b_all, b_d.rearrange("l f -> (l f)").rearrange(
            "(one l f) -> one l f", one=1, l=L))
        gc_all = const.tile([F, L], F32)
        nc.scalar.dma_start(gc_all, g_d.rearrange("l f -> f l"))
        g_row = [g_all[:, l, :] for l in range(L)]
        b_row = [b_all[:, l, :] for l in range(L)]
        g_col = [gc_all[:, l:l + 1] for l in range(L)]
        gb_bf = const.tile([1, 2 * L, F], BF16)
        nc.vector.tensor_copy(gb_bf[:, 0:L, :], g_all)
        nc.vector.tensor_copy(gb_bf[:, L:2 * L, :], b_all)
        g_row_bf = [gb_bf[:, l, :] for l in range(L)]
        b_row_bf = [gb_bf[:, L + l, :] for l in range(L)]

        # ------------- WhT (transposed head weights) + Wtilde = W @ a -------
        WhT_ball = [const.tile([F, H, F], BF16, name=f"WhT{l}") for l in range(L)]
        for l in range(L):
            for h in range(H):
                pt = pou.tile([128, 128], BF16, tag="oU")
                nc.tensor.transpose(pt, Wh_bf[l][h], ident_bf)
                if h % 2 == 0:
                    nc.scalar.activation(WhT_ball[l][:, h, :], pt, AF.Copy)
                else:
                    nc.vector.tensor_copy(WhT_ball[l][:, h, :], pt)
        Wt_bf = [const.tile([F, 2 * H], BF16, name=f"Wt{l}") for l in range(L)]
        for l in range(L):
            pw = prow.tile([128, 2 * H], F32, tag="prow")
            for h in range(H):
                nc.tensor.matmul(pw[:, 2 * h:2 * h + 2], WhT_ball[l][:, h, :],
                                 ah_bf[l][h], start=True, stop=True)
            nc.scalar.activation(Wt_bf[l], pw, AF.Copy)

        # ---------------- x -> xT, input projection ----------------
        xT = const.tile([DIN, N], BF16)
        ph = pmisc.tile([128, N], F32, tag="pbig")
        hT = sbig.tile([128, N], F32, tag="hT")
        hT_bf = sbig.tile([128, N], BF16, tag="hTb", bufs=2)
        for c in range(NCHUNK):
            xb = shd.tile([128, DIN], BF16, tag="xchb")
            nc.vector.tensor_copy(xb, x_chunks[c])
            pt = pmisc.tile([DIN, 128], BF16, tag="pbig")
            nc.tensor.transpose(pt, xb, ident_bf)
            nc.scalar.activation(xT[:, bass.ts(c, 128)], pt, AF.Copy)
            nc.tensor.matmul(ph[:, bass.ts(c, 128)], Wp_sb, xT[:, bass.ts(c, 128)],
                             start=True, stop=True)
            nc.scalar.activation(hT[:, bass.ts(c, 128)], ph[:, bass.ts(c, 128)],
                                 AF.Relu, bias=bp_col)
            nc.vector.tensor_copy(hT_bf[:, bass.ts(c, 128)], hT[:, bass.ts(c, 128)])

        # ---------------- adj -> maskT (bf16, transposed) ----------------
        adj_f = []
        for r in range(NCHUNK):
            ai = shd.tile([128, N], mybir.dt.int32, tag="adji")
            nc.scalar.dma_start(ai, adj_d[bass.ts(r, 128), :])
            af = smask.tile([128, N], BF16, tag="adjf")
            nc.vector.tensor_copy(af, ai)
            adj_f.append(af)
        maskT = [const.tile([128, N], BF16, name=f"maskT{c}") for c in range(NCHUNK)]
        for r in range(NCHUNK):
            for c in range(NCHUNK):
                pm = pmisc.tile([128, 128], BF16, tag="pbig")
                nc.tensor.transpose(pm, adj_f[r][:, bass.ts(c, 128)], ident_bf)
                nc.scalar.activation(maskT[c][:, bass.ts(r, 128)], pm, AF.Copy)

        # ------------- attention body (shared by heads & out-att) -----------
        def attention(e1b_sb, e2i_cols, e2_cols, projNp, hid, out_f32=False):
            """e1b_sb: [128,N] bf16 bcast of E1b row.  e2i_cols/e2_cols: 4
            [128,1] col APs (E2inv f32 / E2 bf16).  projNp: [128,NCHUNK,128]
            bf16 AV lhsT already scaled by E2[m].  Returns outT = pou/den."""
            t_m = sexp.tile([128, NCHUNK, N], BF16, tag="t_m", bufs=3)
            s_t = sexp.tile([128, NCHUNK, N], BF16, tag="s_t", bufs=5)
            for c in range(NCHUNK):
                nc.vector.tensor_scalar_max(t_m[:, c, :], e1b_sb, e2i_cols[c])
                nc.vector.tensor_tensor(s_t[:, c, :], t_m[:, c, :], maskT[c],
                                        OP.mult)
            den_ps = prow.tile([1, N], F32, tag="prow")
            for c in range(NCHUNK):
                nc.tensor.matmul(den_ps, e2_cols[c], s_t[:, c, :],
                                 start=(c == 0), stop=(c == NCHUNK - 1))
            pou_ps = pou.tile([128, N], F32, tag="oU")
            for c in range(NCHUNK):
                nc.tensor.matmul(pou_ps, projNp[:, c, :], s_t[:, c, :],
                                 start=(c == 0), stop=(c == NCHUNK - 1))
            rrow = srow.tile([1, N], F32, tag="rrowf")
            nc.vector.reciprocal_approx_fast(rrow, den_ps)
            rrow_bf = srow.tile([1, N], BF16, tag="rrowb")
            nc.vector.tensor_copy(rrow_bf, rrow)
            rep_ps = prow.tile([128, N], F32, tag="prow")
            nc.tensor.matmul(rep_ps, ones_row_bf[:, 0:128], rrow_bf,
                             start=True, stop=True)
            rep = sbcast.tile([128, N], BF16, tag="rep", bufs=6)
            nc.scalar.activation(rep, rep_ps, AF.Copy)
            pou_bf = shd.tile([128, N], BF16, tag="poubf")
            nc.scalar.activation(pou_bf, pou_ps, AF.Copy)
            outT = sbig.tile([128, N], F32 if out_f32 else BF16, tag="outT",
                             bufs=4)
            nc.vector.tensor_tensor(outT, pou_bf, rep, OP.mult)
            return outT

        # ---------------- layers ----------------
        for l in range(L):
            residT = hT
            # --- rows for all heads: s12[2h] = s1_h, s12[2h+1] = s2_h
            s12_ps = prow.tile([2 * H, N], F32, tag="prow")
            nc.tensor.matmul(s12_ps, Wt_bf[l], hT_bf, start=True, stop=True)
            Eblk = sx.tile([16, N], BF16, tag="Eblk")   # exp(+beta*s): rows 2h = E1b
            nc.scalar.activation(Eblk, s12_ps, AF.Exp, scale=BETA)
            Xneg = sx.tile([16, N], BF16, tag="Xneg")   # exp(-beta*s): 2h+1 = E2inv
            nc.scalar.activation(Xneg, s12_ps, AF.Exp, scale=-BETA)
            Xpos = sx.tile([16, N], BF16, tag="Xpos")   # exp(s): 2h+1 = E2
            nc.scalar.activation(Xpos, s12_ps, AF.Exp, scale=1.0)
            # E1b broadcasts: one DRAM bounce write of all rows, then one
            # stride-0 broadcast read per head, spread across DMA queues
            dmaq = [nc.sync, nc.scalar, nc.gpsimd]
            nc.sync.dma_start(ebl_d[l], Eblk)
            e1b = []
            for h in range(H):
                row = ebl_d[l][2 * h, :]
                src_bc = bass.AP(tensor=row.tensor, offset=row.offset,
                                 ap=[[0, 128], [1, N]])
                eb = sbcast.tile([128, N], BF16, tag="e1b")
                dmaq[h % 3].dma_start(eb, src_bc)
                e1b.append(eb)
            # --- columns: transpose Xneg/Xpos -> Xcols [128, 8*16]
            xc_ps = prow.tile([128, 8 * 16], BF16, tag="prow")
            for c in range(NCHUNK):
                nc.tensor.transpose(xc_ps[:, c * 16:(c + 1) * 16],
                                    Xneg[:, bass.ts(c, 128)], ident_bf[0:16, 0:16])
                nc.tensor.transpose(xc_ps[:, 64 + c * 16:64 + (c + 1) * 16],
                                    Xpos[:, bass.ts(c, 128)], ident_bf[0:16, 0:16])
            Xcols = sx.tile([128, 8 * 16], F32, tag="Xcols")
            nc.scalar.activation(Xcols, xc_ps, AF.Copy)
            Xcols_bf = sx.tile([128, 8 * 16], BF16, tag="Xcolsb")
            nc.vector.tensor_copy(Xcols_bf, xc_ps)

            def e2i_col(h, c):
                j = c * 16 + 2 * h + 1
                return Xcols[:, j:j + 1]

            def e2_col(h, c):
                j = 64 + c * 16 + 2 * h + 1
                return Xcols[:, j:j + 1]

            def e2_col_bf(h, c):
                j = 64 + c * 16 + 2 * h + 1
                return Xcols_bf[:, j:j + 1]

            # --- projN: batched over heads (2 x 512-free MMs per chunk),
            # evacuated per head with E2[m] scale fused
            projNp = [sproj.tile([128, NCHUNK, 128], BF16, tag="projNp",
                                 name=f"pp{l}_{h}") for h in range(H)]
            WhV = Wh_ball[l].rearrange("i h f -> i (h f)")
            for c in range(NCHUNK):
                for g in range(2):
                    pN = pmisc.tile([128, N], F32, tag="pbig")
                    nc.tensor.matmul(pN, hT_bf[:, bass.ts(c, 128)],
                                     WhV[:, bass.ts(g, 512)], start=True, stop=True)
                    for j in range(4):
                        h = g * 4 + j
                        nc.scalar.activation(projNp[h][:, c, :],
                                             pN[:, bass.ts(j, 128)],
                                             AF.Identity, scale=e2_col(h, c))
            # --- attention per head + ELU
            multiT = []
            for h in range(H):
                outT = attention(
                    e1b[h],
                    [e2i_col(h, c) for c in range(NCHUNK)],
                    [e2_col_bf(h, c) for c in range(NCHUNK)],
                    projNp[h], l * 9 + h)
                ex = shd.tile([128, N], BF16, tag="elu_ex")
                nc.scalar.activation(ex, outT, AF.Exp)
                eng = nc.gpsimd if GP_ELU else nc.vector
                eng.tensor_scalar(ex, ex, 1.0, -1.0, OP.min, OP.add)
                mh = smulti.tile([128, N], BF16, tag="multi")
                eng2 = nc.gpsimd if GP_EMAX else nc.vector
                eng2.tensor_tensor(mh, outT, ex, OP.max)
                multiT.append(mh)

            # --- W_out projection
            ph2 = pou.tile([128, N], F32, tag="oU")
            for h in range(H):
                nc.tensor.matmul(ph2, Wo_bf[l][:, h, :], multiT[h],
                                 start=(h == 0), stop=(h == H - 1))
            h2_bf = sbig.tile([128, N], BF16, tag="h2b", bufs=2)
            nc.scalar.activation(h2_bf, ph2, AF.Copy)

            # --- single out-attention
            s12o_ps = prow.tile([2, N], F32, tag="prow")
            nc.tensor.matmul(s12o_ps, ao_bf[l], h2_bf, start=True, stop=True)
            Xo_b = sx.tile([2, N], BF16, tag="Xo_b")    # row 0 = E1b_o
            nc.scalar.activation(Xo_b, s12o_ps, AF.Exp, scale=BETA)
            Xo_nb = sx.tile([2, N], BF16, tag="Xo_nb")  # row 1 = E2inv_o
            nc.scalar.activation(Xo_nb, s12o_ps, AF.Exp, scale=-BETA)
            Xo_1 = sx.tile([2, N], BF16, tag="Xo_1")    # row 1 = E2_o
            nc.scalar.activation(Xo_1, s12o_ps, AF.Exp, scale=1.0)
            xo_ps = prow.tile([128, 16], BF16, tag="prow")
            for c in range(NCHUNK):
                nc.tensor.transpose(xo_ps[:, c * 2:(c + 1) * 2],
                                    Xo_nb[:, bass.ts(c, 128)], ident_bf[0:2, 0:2])
                nc.tensor.transpose(xo_ps[:, 8 + c * 2:8 + (c + 1) * 2],
                                    Xo_1[:, bass.ts(c, 128)], ident_bf[0:2, 0:2])
            Xoc = sx.tile([128, 16], F32, tag="Xoc")
            nc.scalar.activation(Xoc, xo_ps, AF.Copy)
            Xoc_bf = sx.tile([128, 16], BF16, tag="Xocb")
            nc.vector.tensor_copy(Xoc_bf, xo_ps)
            # E1b_o broadcast via PE rank-1 (low latency; PE idle here)
            ebo_ps = pmisc.tile([128, N], F32, tag="pbig")
            nc.tensor.matmul(ebo_ps, ones_row_bf[:, 0:128], Xo_b[0:1, :], start=True, stop=True)
            e1bo = sbcast.tile([128, N], BF16, tag="e1b")
            nc.scalar.activation(e1bo, ebo_ps, AF.Copy)
            # h2N via transposes, scaled by E2o[m]
            h2n_ps = pmisc.tile([128, N], BF16, tag="pbig")
            for c in range(NCHUNK):
                nc.tensor.transpose(h2n_ps[:, bass.ts(c, 128)],
                                    h2_bf[:, bass.ts(c, 128)], ident_bf)
            h2Np = sproj.tile([128, NCHUNK, 128], BF16, tag="projNp")
            for c in range(NCHUNK):
                nc.scalar.activation(h2Np[:, c, :], h2n_ps[:, bass.ts(c, 128)],
                                     AF.Identity, scale=Xoc[:, 8 + c * 2 + 1:8 + c * 2 + 2])
            outsT = attention(
                e1bo,
                [Xoc[:, c * 2 + 1:c * 2 + 2] for c in range(NCHUNK)],
                [Xoc_bf[:, 8 + c * 2 + 1:8 + c * 2 + 2] for c in range(NCHUNK)],
                h2Np, l * 9 + 8, out_f32=True)

            # ---- residual + LN over partition dim ----
            xs = sbig.tile([128, N], F32, tag="xs", bufs=2)
            nc.vector.tensor_tensor(xs, outsT, residT, OP.add)
            xs_bf = sbig.tile([128, N], BF16, tag="xsqb", bufs=2)
            nc.vector.tensor_copy(xs_bf, xs)
            xsq = sbig.tile([128, N], BF16, tag="xsq", bufs=2)
            nc.scalar.activation(xsq, xs, AF.Square)
            pmu = prow.tile([1, N], F32, tag="prow")
            nc.tensor.matmul(pmu, ones_col_bf, xs_bf, start=True, stop=True)
            psq = prow.tile([1, N], F32, tag="prow")
            nc.tensor.matmul(psq, ones_col_bf, xsq, start=True, stop=True)
            mu = srow.tile([1, N], F32, tag="rowL")
            nc.vector.tensor_scalar_mul(mu, pmu, 1.0 / F)
            msq = srow.tile([1, N], F32, tag="rowL")
            nc.vector.tensor_scalar_mul(msq, psq, 1.0 / F)
            mu2 = srow.tile([1, N], F32, tag="rowL")
            nc.vector.tensor_tensor(mu2, mu, mu, OP.mult)
            var = srow.tile([1, N], F32, tag="rowL")
            nc.vector.tensor_tensor(var, msq, mu2, OP.subtract)
            lnv = srow.tile([1, N], F32, tag="rowL")
            nc.scalar.activation(lnv, var, AF.Ln, bias=eps1)
            rstd = srow.tile([1, N], F32, tag="rowL")
            nc.scalar.activation(rstd, lnv, AF.Exp, scale=-0.5)
            mr = srow.tile([1, N], F32, tag="rowL")
            nc.vector.tensor_tensor(mr, mu, rstd, OP.mult)
            r2 = srow.tile([1, N], BF16, tag="rowLb")
            nc.vector.tensor_scalar_mul(r2, mr, -1.0)
            rstd_bf = srow.tile([1, N], BF16, tag="rowLb")
            nc.vector.tensor_copy(rstd_bf, rstd)
            paff = pmisc.tile([128, N], F32, tag="pbig")
            nc.tensor.matmul(paff, g_row_bf[l], r2, start=True, stop=False)
            nc.tensor.matmul(paff, b_row_bf[l], ones_row_bf, start=False, stop=True)
            prs = pmisc.tile([128, N], F32, tag="pbig")
            nc.tensor.matmul(prs, g_row_bf[l], rstd_bf,
                             start=True, stop=True)
            rep_grstd = sbig.tile([128, N], F32, tag="repo", bufs=2)
            nc.scalar.activation(rep_grstd, prs, AF.Copy)
            y = sbig.tile([128, N], F32, tag="y", bufs=2)
            nc.vector.tensor_tensor(y, xs, rep_grstd, OP.mult)
            hT_new = sbig.tile([128, N], F32, tag="hT")
            nc.vector.tensor_tensor(hT_new, y, paff, OP.add)
            if l < L - 1:
                nc.vector.tensor_scalar_max(hT_new, hT_new, 0.0)
            hT = hT_new
            if l < L - 1:
                hT_bf = sbig.tile([128, N], BF16, tag="hTb", bufs=2)
                nc.vector.tensor_copy(hT_bf, hT)

        # ---------------- output: transpose back ----------------
        for c in range(NCHUNK):
            po = pmisc.tile([128, 128], F32, tag="pbig")
            nc.tensor.transpose(po, hT[:, bass.ts(c, 128)], ident)
            osb = shd.tile([128, 128], F32, tag="osb")
            nc.scalar.activation(osb, po, AF.Copy)
            nc.sync.dma_start(out_d[bass.ts(c, 128), :], osb)

    nc.compile()
    return nc


def _get_nc():
    if "nc" not in _CACHE:
        _CACHE["nc"] = build_nc()
    return _CACHE["nc"]


def kernel(**inputs) -> np.ndarray:
    nc = _get_nc()
    shared = {k: np.ascontiguousarray(np.asarray(inputs[k], dtype=np.float32))
              for k in ("Wp", "bp", "W_heads", "a_heads", "W_out", "a_out",
                        "ln_g", "ln_b")}
    x = np.asarray(inputs["x"], dtype=np.float32)
    adj = np.asarray(inputs["adj"], dtype=np.int32)
    in_maps = [dict(x=np.ascontiguousarray(x[b]),
                    adj=np.ascontiguousarray(adj[b]), **shared)
               for b in range(B)]
    res = run_bass_kernel_spmd(nc, in_maps, core_ids=list(range(B)))
    return np.stack([res.results[b]["out"] for b in range(B)])


if __name__ == "__main__":
    rng = np.random.default_rng(0)
    inputs = dict(
        x=rng.normal(size=(B, N, DIN)).astype(np.float32),
        adj=rng.integers(0, 2, size=(B, N, N)).astype(np.int32),
        Wp=(rng.normal(size=(DIN, F)) * 0.12).astype(np.float32),
        bp=np.zeros(F, dtype=np.float32),
        W_heads=(rng.normal(size=(L, H, F, F)) * 0.08).astype(np.float32),
        a_heads=(rng.normal(size=(L, H, 2 * F)) * 0.08).astype(np.float32),
        W_out=(rng.normal(size=(L, H * F, F)) * 0.03).astype(np.float32),
        a_out=(rng.normal(size=(L, 2 * F)) * 0.08).astype(np.float32),
        ln_g=np.ones((L, F), dtype=np.float32),
        ln_b=np.zeros((L, F), dtype=np.float32),
    )
    out = kernel(**inputs)
    print("out", out.shape, out.dtype, np.abs(out).max())


# revision 17
# speedup vs baseline: 1.0770x; 1.0054x over previous
"""GAT spatio-temporal model Trainium2 kernel (v4).

Sharding: data-parallel over batch B=8 -> 8 NeuronCores (1 graph each).

v4 core trick: exp(leaky_relu(s1[n]+s2[m])) = max(E1*E2, E1a*E2a) with
E = exp(s), Ea = exp(alpha*s) (exp monotone, lrelu(x) = max(x, a*x)).
Factor p = E1a[n] * E2[m] * max(E1b[n], E2inv[m]) with E1b = exp((1-a)s1),
E2inv = exp(-(1-a)s2).  E1a[n] is constant along the softmax axis (m) and
cancels; E2[m] folds into the den / AV matmul lhsT weights.  The whole
[N,N] attention tensor is then ONE fused DVE op per 128-chunk:
scalar_tensor_tensor(out, E1b_bcast, E2inv_col, maskT, max, mult).
No N^2 ScalarE work; s1/s2 come from one matmul via precomposed W@a.
All N^2 matmuls bf16; LN matmuls float32r.

Shapes (hardcoded): B=8, N=512, Din=64, H=8, F=128, L=2.
"""
import os
import numpy as np
from contextlib import ExitStack

import concourse.bass as bass
import concourse.tile as tile
from concourse import bacc, mybir
from concourse.bass_utils import run_bass_kernel_spmd
from concourse.masks import make_identity

F32 = mybir.dt.float32
F32R = mybir.dt.float32r
BF16 = mybir.dt.bfloat16
AF = mybir.ActivationFunctionType
OP = mybir.AluOpType

B, N, DIN, H, F, L = 8, 512, 64, 8, 128, 2
NCHUNK = N // 128  # 4
ALPHA = 0.2
BETA = 1.0 - ALPHA
LN_EPS = 1e-5

GP_STT = int(os.environ.get("K_GP_STT", "0"))   # STT chunks on gpsimd
GP_ELU = os.environ.get("K_GP_ELU", "0") == "1"  # ELU tensor_scalar on gpsimd
GP_EMAX = os.environ.get("K_GP_EMAX", "0") == "1"  # ELU max on gpsimd
GP_PSC = int(os.environ.get("K_GP_PSC", "0"))   # projNp scale chunks on gpsimd

_CACHE = {}


def _bcast_row(ap_row):
    return bass.AP(tensor=ap_row.tensor, offset=ap_row.offset, ap=[[0, 128], [1, N]])


def _r(ap):
    return ap.bitcast(F32R)


def build_nc():
    nc = bacc.Bacc("TRN2", target_bir_lowering=False, debug=False)

    x_d = nc.dram_tensor("x", [N, DIN], F32, kind="ExternalInput").ap()
    adj_d = nc.dram_tensor("adj", [N, N], mybir.dt.int32, kind="ExternalInput").ap()
    Wp_d = nc.dram_tensor("Wp", [DIN, F], F32, kind="ExternalInput").ap()
    bp_d = nc.dram_tensor("bp", [F], F32, kind="ExternalInput").ap()
    Wh_d = nc.dram_tensor("W_heads", [L, H, F, F], F32, kind="ExternalInput").ap()
    ah_d = nc.dram_tensor("a_heads", [L, H, 2 * F], F32, kind="ExternalInput").ap()
    Wo_d = nc.dram_tensor("W_out", [L, H * F, F], F32, kind="ExternalInput").ap()
    ao_d = nc.dram_tensor("a_out", [L, 2 * F], F32, kind="ExternalInput").ap()
    g_d = nc.dram_tensor("ln_g", [L, F], F32, kind="ExternalInput").ap()
    b_d = nc.dram_tensor("ln_b", [L, F], F32, kind="ExternalInput").ap()
    out_d = nc.dram_tensor("out", [N, F], F32, kind="ExternalOutput").ap()
    # DRAM bounce buffers: E1b rows (per layer) + per-head recip rows
    ebl_d = [nc.dram_tensor(f"eblk{l}", [16, N], BF16, kind="ExternalOutput").ap()
             for l in range(L)]
    scr_d = [nc.dram_tensor(f"scratch{i}", [1, N], BF16, kind="ExternalOutput").ap()
             for i in range(18)]

    with tile.TileContext(nc) as tc, ExitStack() as ctx:
        const = ctx.enter_context(tc.tile_pool(name="const", bufs=1))
        sx = ctx.enter_context(tc.tile_pool(name="sx", bufs=2))
        sproj = ctx.enter_context(tc.tile_pool(name="sproj", bufs=10))
        sbcast = ctx.enter_context(tc.tile_pool(name="sbcast", bufs=10))
        sexp = ctx.enter_context(tc.tile_pool(name="sexp", bufs=4))
        smulti = ctx.enter_context(tc.tile_pool(name="smulti", bufs=9))
        sbig = ctx.enter_context(tc.tile_pool(name="sbig", bufs=3))
        srow = ctx.enter_context(tc.tile_pool(name="srow", bufs=5))
        shd = ctx.enter_context(tc.tile_pool(name="shd", bufs=4))
        smask = ctx.enter_context(tc.tile_pool(name="smask", bufs=4))
        pou = ctx.enter_context(tc.tile_pool(name="pou", bufs=3, space="PSUM"))
        pmisc = ctx.enter_context(tc.tile_pool(name="pmisc", bufs=2, space="PSUM"))
        prow = ctx.enter_context(tc.tile_pool(name="prow", bufs=3, space="PSUM"))

        # ---------------- constants ----------------
        ones_row = const.tile([1, N], F32)
        nc.vector.memset(ones_row, 1.0)
        ones_row_bf = const.tile([1, N], BF16)
        nc.vector.memset(ones_row_bf, 1.0)
        ones_col = const.tile([128, 1], F32)
        nc.vector.memset(ones_col, 1.0)
        ones_col_bf = const.tile([128, 1], BF16)
        nc.vector.memset(ones_col_bf, 1.0)
        ident = const.tile([128, 128], F32)
        make_identity(nc, ident)
        ident_bf = const.tile([128, 128], BF16)
        nc.vector.tensor_copy(ident_bf, ident)
        eps1 = const.tile([1, 1], F32)
        nc.vector.memset(eps1, LN_EPS)

        Wp_sb = const.tile([DIN, F], BF16)
        nc.gpsimd.dma_start(Wp_sb, Wp_d)
        bp_col = const.tile([F, 1], F32)
        nc.sync.dma_start(bp_col, bp_d.rearrange("(f one) -> f one", one=1))
        x_chunks = []
        for c in range(NCHUNK):
            xc = shd.tile([128, DIN], F32, tag="xchunk")
            nc.sync.dma_start(xc, x_d[bass.ts(c, 128), :])
            x_chunks.append(xc)

        # layer-0 head weights on fast HW queue (fp32) + DVE cast; the rest
        # trickle in on the gpsimd software queue with cast
        ah_ball = const.tile([F, L * H, 2], BF16)
        nc.gpsimd.dma_start(ah_ball, ah_d.rearrange("l h (t f) -> f (l h) t", t=2))
        ah_bf = [[ah_ball[:, l * H + h, :] for h in range(H)] for l in range(L)]

        Wh_ball = [const.tile([F, H, F], BF16, name=f"WhB{l}") for l in range(L)]
        Wh0_f = const.tile([F, H, F], F32)
        nc.sync.dma_start(Wh0_f, Wh_d[0].rearrange("h i o -> i h o"))
        nc.vector.tensor_copy(Wh_ball[0], Wh0_f)
        nc.gpsimd.dma_start(Wh_ball[1], Wh_d[1].rearrange("h i o -> i h o"))
        Wh_bf = [[Wh_ball[l][:, h, :] for h in range(H)] for l in range(L)]

        Wo_ball = [const.tile([128, H, F], BF16, name=f"WoB{l}") for l in range(L)]
        for l in range(L):
            nc.gpsimd.dma_start(Wo_ball[l], Wo_d[l].rearrange("(c p) f -> p c f", p=128))
        Wo_bf = Wo_ball

        ao_ball = const.tile([F, L, 2], BF16)
        nc.gpsimd.dma_start(ao_ball, ao_d.rearrange("l (t f) -> f l t", t=2))
        ao_bf = [ao_ball[:, l, :] for l in range(L)]

        g_all = const.tile([1, L, F], F32)
        nc.scalar.dma_start(g_all, g_d.rearrange("l f -> (l f)").rearrange(
            "(one l f) -> one l f", one=1, l=L))
        b_all = const.tile([1, L, F], F32)
        nc.scalar.dma_start(b_all, b_d.rearrange("l f -> (l f)").rearrange(
            "(one l f) -> one l f", one=1, l=L))
        gc_all = const.tile([F, L], F32)
        nc.scalar.dma_start(gc_all, g_d.rearrange("l f -> f l"))
        g_row = [g_all[:, l, :] for l in range(L)]
        b_row = [b_all[:, l, :] for l in range(L)]
        g_col = [gc_all[:, l:l + 1] for l in range(L)]
        gb_bf = const.tile([1, 2 * L, F], BF16)
        nc.vector.tensor_copy(gb_bf[:, 0:L, :], g_all)
        nc.vector.tensor_copy(gb_bf[:, L:2 * L, :], b_all)
        g_row_bf = [gb_bf[:, l, :] for l in range(L)]
        b_row_bf = [gb_bf[:, L + l, :] for l in range(L)]

        # ------------- WhT (transposed head weights) + Wtilde = W @ a -------
        WhT_ball = [const.tile([F, H, F], BF16, name=f"WhT{l}") for l in range(L)]
        for l in range(L):
            for h in range(H):
                pt = pou.tile([128, 128], BF16, tag="oU")
                nc.tensor.transpose(pt, Wh_bf[l][h], ident_bf)
                if h % 2 == 0:
                    nc.scalar.activation(WhT_ball[l][:, h, :], pt, AF.Copy)
                else:
                    nc.vector.tensor_copy(WhT_ball[l][:, h, :], pt)
        Wt_bf = [const.tile([F, 2 * H], BF16, name=f"Wt{l}") for l in range(L)]
        for l in range(L):
            pw = prow.tile([128, 2 * H], F32, tag="prow")
            for h in range(H):
                nc.tensor.matmul(pw[:, 2 * h:2 * h + 2], WhT_ball[l][:, h, :],
                                 ah_bf[l][h], start=True, stop=True)
            nc.scalar.activation(Wt_bf[l], pw, AF.Copy)

        # ---------------- x -> xT, input projection ----------------
        xT = const.tile([DIN, N], BF16)
        ph = pmisc.tile([128, N], F32, tag="pbig")
        hT = sbig.tile([128, N], F32, tag="hT")
        hT_bf = sbig.tile([128, N], BF16, tag="hTb", bufs=2)
        for c in range(NCHUNK):
            xb = shd.tile([128, DIN], BF16, tag="xchb")
            nc.vector.tensor_copy(xb, x_chunks[c])
            pt = pmisc.tile([DIN, 128], BF16, tag="pbig")
            nc.tensor.transpose(pt, xb, ident_bf)
            nc.scalar.activation(xT[:, bass.ts(c, 128)], pt, AF.Copy)
            nc.tensor.matmul(ph[:, bass.ts(c, 128)], Wp_sb, xT[:, bass.ts(c, 128)],
                             start=True, stop=True)
            nc.scalar.activation(hT[:, bass.ts(c, 128)], ph[:, bass.ts(c, 128)],
                                 AF.Relu, bias=bp_col)
            nc.vector.tensor_copy(hT_bf[:, bass.ts(c, 128)], hT[:, bass.ts(c, 128)])

        # ---------------- adj -> maskT (bf16, transposed) ----------------
        adj_f = []
        for r in range(NCHUNK):
            ai = shd.tile([128, N], mybir.dt.int32, tag="adji")
            nc.scalar.dma_start(ai, adj_d[bass.ts(r, 128), :])
            af = smask.tile([128, N], BF16, tag="adjf")
            nc.vector.tensor_copy(af, ai)
            adj_f.append(af)
        maskT = [const.tile([128, N], BF16, name=f"maskT{c}") for c in range(NCHUNK)]
        for r in range(NCHUNK):
            for c in range(NCHUNK):
                pm = pmisc.tile([128, 128], BF16, tag="pbig")
                nc.tensor.transpose(pm, adj_f[r][:, bass.ts(c, 128)], ident_bf)
                nc.scalar.activation(maskT[c][:, bass.ts(r, 128)], pm, AF.Copy)

        # ------------- attention body (shared by heads & out-att) -----------
        def attention(e1b_sb, e2i_cols, e2_cols, projNp, hid, out_f32=False):
            """e1b_sb: [128,N] bf16 bcast of E1b row.  e2i_cols/e2_cols: 4
            [128,1] col APs (E2inv f32 / E2 bf16).  projNp: [128,NCHUNK,128]
            bf16 AV lhsT already scaled by E2[m].  Returns outT = pou/den."""
            t_m = sexp.tile([128, NCHUNK, N], BF16, tag="t_m", bufs=3)
            s_t = sexp.tile([128, NCHUNK, N], BF16, tag="s_t", bufs=5)
            for c in range(NCHUNK):
                nc.vector.tensor_scalar_max(t_m[:, c, :], e1b_sb, e2i_cols[c])
                nc.vector.tensor_tensor(s_t[:, c, :], t_m[:, c, :], maskT[c],
                                        OP.mult)
            den_ps = prow.tile([1, N], F32, tag="prow")
            for c in range(NCHUNK):
                nc.tensor.matmul(den_ps, e2_cols[c], s_t[:, c, :],
                                 start=(c == 0), stop=(c == NCHUNK - 1))
            pou_ps = pou.tile([128, N], F32, tag="oU")
            for c in range(NCHUNK):
                nc.tensor.matmul(pou_ps, projNp[:, c, :], s_t[:, c, :],
                                 start=(c == 0), stop=(c == NCHUNK - 1))
            rrow = srow.tile([1, N], F32, tag="rrowf")
            nc.vector.reciprocal_approx_fast(rrow, den_ps)
            rrow_bf = srow.tile([1, N], BF16, tag="rrowb")
            nc.vector.tensor_copy(rrow_bf, rrow)
            rep_ps = prow.tile([128, N], F32, tag="prow")
            nc.tensor.matmul(rep_ps, ones_row_bf[:, 0:128], rrow_bf,
                             start=True, stop=True)
            rep = sbcast.tile([128, N], BF16, tag="rep", bufs=6)
            nc.scalar.activation(rep, rep_ps, AF.Copy)
            pou_bf = shd.tile([128, N], BF16, tag="poubf")
            nc.scalar.activation(pou_bf, pou_ps, AF.Copy)
            outT = sbig.tile([128, N], F32 if out_f32 else BF16, tag="outT",
                             bufs=4)
            nc.vector.tensor_tensor(outT, pou_bf, rep, OP.mult)
            return outT

        # ---------------- layers ----------------
        for l in range(L):
            residT = hT
            # --- rows for all heads: s12[2h] = s1_h, s12[2h+1] = s2_h
            s12_ps = prow.tile([2 * H, N], F32, tag="prow")
            nc.tensor.matmul(s12_ps, Wt_bf[l], hT_bf, start=True, stop=True)
            Eblk = sx.tile([16, N], BF16, tag="Eblk")   # exp(+beta*s): rows 2h = E1b
            nc.scalar.activation(Eblk, s12_ps, AF.Exp, scale=BETA)
            # E1b broadcasts: one DRAM bounce write of all rows, then one
            # stride-0 broadcast read per head, spread across DMA queues
            dmaq = [nc.sync, nc.scalar, nc.gpsimd]
            nc.sync.dma_start(ebl_d[l], Eblk)
            e1b = []
            for h in range(H):
                row = ebl_d[l][2 * h, :]
                src_bc = bass.AP(tensor=row.tensor, offset=row.offset,
                                 ap=[[0, 128], [1, N]])
                eb = sbcast.tile([128, N], BF16, tag="e1b")
                dmaq[h % 3].dma_start(eb, src_bc)
                e1b.append(eb)
            # --- s2 columns directly via tiny matmuls (no transposes)
            Wt2 = Wt_bf[l].rearrange("i (h t) -> i t h", t=2)[:, 1, :]
            cps = prow.tile([128, NCHUNK, 8], F32, tag="prow")
            for c in range(NCHUNK):
                nc.tensor.matmul(cps[:, c, :], hT_bf[:, bass.ts(c, 128)], Wt2,
                                 start=True, stop=True)
            C_e2i = sx.tile([128, NCHUNK, 8], F32, tag="Ce2i")
            nc.scalar.activation(C_e2i, cps, AF.Exp, scale=-BETA)
            C_e2b = sx.tile([128, NCHUNK, 8], BF16, tag="Ce2b")
            nc.scalar.activation(C_e2b, cps, AF.Exp, scale=1.0)
            C_e2f = sx.tile([128, NCHUNK, 8], F32, tag="Ce2f")
            nc.scalar.activation(C_e2f, cps, AF.Exp, scale=1.0)

            def e2i_col(h, c):
                return C_e2i[:, c, h:h + 1]

            def e2_col(h, c):
                return C_e2f[:, c, h:h + 1]

            def e2_col_bf(h, c):
                return C_e2b[:, c, h:h + 1]

            # --- projN: batched over heads (2 x 512-free MMs per chunk),
            # evacuated per head with E2[m] scale fused
            projNp = [sproj.tile([128, NCHUNK, 128], BF16, tag="projNp",
                                 name=f"pp{l}_{h}") for h in range(H)]
            WhV = Wh_ball[l].rearrange("i h f -> i (h f)")
            for c in range(NCHUNK):
                for g in range(2):
                    pN = pmisc.tile([128, N], F32, tag="pbig")
                    nc.tensor.matmul(pN, hT_bf[:, bass.ts(c, 128)],
                                     WhV[:, bass.ts(g, 512)], start=True, stop=True)
                    for j in range(4):
                        h = g * 4 + j
                        nc.scalar.activation(projNp[h][:, c, :],
                                             pN[:, bass.ts(j, 128)],
                                             AF.Identity, scale=e2_col(h, c))
            # --- attention per head + ELU
            multiT = []
            for h in range(H):
                outT = attention(
                    e1b[h],
                    [e2i_col(h, c) for c in range(NCHUNK)],
                    [e2_col_bf(h, c) for c in range(NCHUNK)],
                    projNp[h], l * 9 + h)
                ex = shd.tile([128, N], BF16, tag="elu_ex")
                nc.scalar.activation(ex, outT, AF.Exp)
                eng = nc.gpsimd if GP_ELU else nc.vector
                eng.tensor_scalar(ex, ex, 1.0, -1.0, OP.min, OP.add)
                mh = smulti.tile([128, N], BF16, tag="multi")
                eng2 = nc.gpsimd if GP_EMAX else nc.vector
                eng2.tensor_tensor(mh, outT, ex, OP.max)
                multiT.append(mh)

            # --- W_out projection
            ph2 = pou.tile([128, N], F32, tag="oU")
            for h in range(H):
                nc.tensor.matmul(ph2, Wo_bf[l][:, h, :], multiT[h],
                                 start=(h == 0), stop=(h == H - 1))
            h2_bf = sbig.tile([128, N], BF16, tag="h2b", bufs=2)
            nc.scalar.activation(h2_bf, ph2, AF.Copy)

            # --- single out-attention
            s12o_ps = prow.tile([2, N], F32, tag="prow")
            nc.tensor.matmul(s12o_ps, ao_bf[l], h2_bf, start=True, stop=True)
            Xo_b = sx.tile([1, N], BF16, tag="Xo_b")    # E1b_o row
            nc.scalar.activation(Xo_b, s12o_ps[0:1, :], AF.Exp, scale=BETA)
            so_ps = prow.tile([128, NCHUNK, 2], F32, tag="prow")
            for c in range(NCHUNK):
                nc.tensor.matmul(so_ps[:, c, :], h2_bf[:, bass.ts(c, 128)],
                                 ao_bf[l], start=True, stop=True)
            Co_e2i = sx.tile([128, NCHUNK, 2], F32, tag="Coe2i")
            nc.scalar.activation(Co_e2i, so_ps, AF.Exp, scale=-BETA)
            Co_e2b = sx.tile([128, NCHUNK, 2], BF16, tag="Coe2b")
            nc.scalar.activation(Co_e2b, so_ps, AF.Exp, scale=1.0)
            Co_e2f = sx.tile([128, NCHUNK, 2], F32, tag="Coe2f")
            nc.scalar.activation(Co_e2f, so_ps, AF.Exp, scale=1.0)
            # E1b_o broadcast via PE rank-1 (low latency; PE idle here)
            ebo_ps = pmisc.tile([128, N], F32, tag="pbig")
            nc.tensor.matmul(ebo_ps, ones_row_bf[:, 0:128], Xo_b, start=True, stop=True)
            e1bo = sbcast.tile([128, N], BF16, tag="e1b")
            nc.scalar.activation(e1bo, ebo_ps, AF.Copy)
            # h2N via transposes, scaled by E2o[m]
            h2n_ps = pmisc.tile([128, N], BF16, tag="pbig")
            for c in range(NCHUNK):
                nc.tensor.transpose(h2n_ps[:, bass.ts(c, 128)],
                                    h2_bf[:, bass.ts(c, 128)], ident_bf)
            h2Np = sproj.tile([128, NCHUNK, 128], BF16, tag="projNp")
            for c in range(NCHUNK):
                nc.scalar.activation(h2Np[:, c, :], h2n_ps[:, bass.ts(c, 128)],
                                     AF.Identity, scale=Co_e2f[:, c, 1:2])
            outsT = attention(
                e1bo,
                [Co_e2i[:, c, 1:2] for c in range(NCHUNK)],
                [Co_e2b[:, c, 1:2] for c in range(NCHUNK)],
                h2Np, l * 9 + 8, out_f32=True)

            # ---- residual + LN over partition dim ----
            xs = sbig.tile([128, N], F32, tag="xs", bufs=2)
            nc.vector.tensor_tensor(xs, outsT, residT, OP.add)
            xs_bf = sbig.tile([128, N], BF16, tag="xsqb", bufs=2)
            nc.vector.tensor_copy(xs_bf, xs)
            xsq = sbig.tile([128, N], BF16, tag="xsq", bufs=2)
            nc.scalar.activation(xsq, xs, AF.Square)
            pmu = prow.tile([1, N], F32, tag="prow")
            nc.tensor.matmul(pmu, ones_col_bf, xs_bf, start=True, stop=True)
            psq = prow.tile([1, N], F32, tag="prow")
            nc.tensor.matmul(psq, ones_col_bf, xsq, start=True, stop=True)
            mu = srow.tile([1, N], F32, tag="rowL")
            nc.vector.tensor_scalar_mul(mu, pmu, 1.0 / F)
            msq = srow.tile([1, N], F32, tag="rowL")
            nc.vector.tensor_scalar_mul(msq, psq, 1.0 / F)
            mu2 = srow.tile([1, N], F32, tag="rowL")
            nc.vector.tensor_tensor(mu2, mu, mu, OP.mult)
            var = srow.tile([1, N], F32, tag="rowL")
            nc.vector.tensor_tensor(var, msq, mu2, OP.subtract)
            lnv = srow.tile([1, N], F32, tag="rowL")
            nc.scalar.activation(lnv, var, AF.Ln, bias=eps1)
            rstd = srow.tile([1, N], F32, tag="rowL")
            nc.scalar.activation(rstd, lnv, AF.Exp, scale=-0.5)
            r2 = srow.tile([1, N], BF16, tag="rowLb")
            nc.vector.scalar_tensor_tensor(r2, mu, -1.0, rstd, OP.mult, OP.mult)
            rstd_bf = srow.tile([1, N], BF16, tag="rowLb")
            nc.vector.tensor_copy(rstd_bf, rstd)
            paff = pmisc.tile([128, N], F32, tag="pbig")
            nc.tensor.matmul(paff, g_row_bf[l], r2, start=True, stop=False)
            nc.tensor.matmul(paff, b_row_bf[l], ones_row_bf, start=False, stop=True)
            prs = pmisc.tile([128, N], F32, tag="pbig")
            nc.tensor.matmul(prs, g_row_bf[l], rstd_bf,
                             start=True, stop=True)
            rep_grstd = sbig.tile([128, N], F32, tag="repo", bufs=2)
            nc.scalar.activation(rep_grstd, prs, AF.Copy)
            y = sbig.tile([128, N], F32, tag="y", bufs=2)
            nc.vector.tensor_tensor(y, xs, rep_grstd, OP.mult)
            hT_new = sbig.tile([128, N], F32, tag="hT")
            nc.vector.tensor_tensor(hT_new, y, paff, OP.add)
            if l < L - 1:
                nc.vector.tensor_scalar_max(hT_new, hT_new, 0.0)
            hT = hT_new
            if l < L - 1:
                hT_bf = sbig.tile([128, N], BF16, tag="hTb", bufs=2)
                nc.vector.tensor_copy(hT_bf, hT)

        # ---------------- output: transpose back ----------------
        for c in range(NCHUNK):
            po = pmisc.tile([128, 128], F32, tag="pbig")
            nc.tensor.transpose(po, hT[:, bass.ts(c, 128)], ident)
            osb = shd.tile([128, 128], F32, tag="osb")
            nc.scalar.activation(osb, po, AF.Copy)
            nc.sync.dma_start(out_d[bass.ts(c, 128), :], osb)

    nc.compile()
    return nc


def _get_nc():
    if "nc" not in _CACHE:
        _CACHE["nc"] = build_nc()
    return _CACHE["nc"]


def kernel(**inputs) -> np.ndarray:
    nc = _get_nc()
    shared = {k: np.ascontiguousarray(np.asarray(inputs[k], dtype=np.float32))
              for k in ("Wp", "bp", "W_heads", "a_heads", "W_out", "a_out",
                        "ln_g", "ln_b")}
    x = np.asarray(inputs["x"], dtype=np.float32)
    adj = np.asarray(inputs["adj"], dtype=np.int32)
    in_maps = [dict(x=np.ascontiguousarray(x[b]),
                    adj=np.ascontiguousarray(adj[b]), **shared)
               for b in range(B)]
    res = run_bass_kernel_spmd(nc, in_maps, core_ids=list(range(B)))
    return np.stack([res.results[b]["out"] for b in range(B)])


if __name__ == "__main__":
    rng = np.random.default_rng(0)
    inputs = dict(
        x=rng.normal(size=(B, N, DIN)).astype(np.float32),
        adj=rng.integers(0, 2, size=(B, N, N)).astype(np.int32),
        Wp=(rng.normal(size=(DIN, F)) * 0.12).astype(np.float32),
        bp=np.zeros(F, dtype=np.float32),
        W_heads=(rng.normal(size=(L, H, F, F)) * 0.08).astype(np.float32),
        a_heads=(rng.normal(size=(L, H, 2 * F)) * 0.08).astype(np.float32),
        W_out=(rng.normal(size=(L, H * F, F)) * 0.03).astype(np.float32),
        a_out=(rng.normal(size=(L, 2 * F)) * 0.08).astype(np.float32),
        ln_g=np.ones((L, F), dtype=np.float32),
        ln_b=np.zeros((L, F), dtype=np.float32),
    )
    out = kernel(**inputs)
    print("out", out.shape, out.dtype, np.abs(out).max())
